# revision 6
# baseline (speedup 1.0000x reference)
"""MeshFC kernel v2 for 8x TRN2 NeuronCores.

Computes: out = inputs @ w + biases, where
  w[i,o] = ||in_pos[i]-out_pos[o]|| - ||init_in_pos[i]-init_out_pos[o]||

Sharding: tensor-parallel on the output dim (8 x 1024 columns).

v2 changes vs v1:
- weight gen via hi/lo split-precision fp16 matmuls (K=21) instead of fp32
  (4 cyc/row -> 1 cyc/row): dist^2 = sum of exact fp16 products
  (hi*hi + hi*lo + lo*hi), fp32-quality at 16-bit speed. One ScalarE
  sqrt(x+1e-6) per tile replaces the clamp+sqrt+sqrt chain; main groups
  ot0/ot1 are interleaved into the wgen program order so the in-order PE
  queue never idles behind the vector chain.
- main matmul is w-stationary: stationary [128k x 128o] weight block reused
  across 4 moving x-tiles [128k x 512b]; redundant LDWEIGHTS are stripped
  post-compile (matmults are non-self-loading). Output tiles land as
  [o=partition, b=free] so bias is a per-partition scalar folded into the
  DVE PSUM-drain (tensor_scalar_add) - no bias matmuls.
- out written fp16 (halves out DMA), host transposes/concats to [B, O] fp32.
"""

import os
from contextlib import ExitStack

import numpy as np

NUM_IN, NUM_OUT, SD, BATCH = 2048, 8192, 5, 4096
N_CORES = 8
O_SHARD = NUM_OUT // N_CORES  # 1024
K_TILES = NUM_IN // 128  # 16
OT = O_SHARD // 128  # 8 o-tiles per core
BG = 4  # batch groups of 512 accumulating simultaneously (4 PSUM banks)
BH = BATCH // (BG * 512)  # 2 passes over the batch
KAUG = 21  # 7 aug rows x 3 (hi*hi, hi*lo, lo*hi)
AUG_W = 2 * NUM_IN + 2 * O_SHARD  # [LC | LX | RC | RX]

_CACHE = {}


def _build_bass(variant=""):
    import concourse.bass as bass  # noqa: F401
    import concourse.mybir as mybir
    from concourse import bacc
    from concourse.tile import TileContext

    fp32 = mybir.dt.float32
    fp16 = mybir.dt.float16

    nc = bacc.Bacc("TRN2", name="meshfc2")

    xT = nc.dram_tensor("xT", [K_TILES, 128, BATCH], fp16, kind="ExternalInput")
    aug = nc.dram_tensor("aug", [KAUG, AUG_W], fp16, kind="ExternalInput")
    bias = nc.dram_tensor("bias", [128, OT], fp32, kind="ExternalInput")
    out = nc.dram_tensor("out", [O_SHARD, BATCH], fp16, kind="ExternalOutput")

    n_rep = 1
    for tok in variant.split(","):
        if tok.startswith("rep"):
            n_rep = int(tok[3:])

    with ExitStack() as ctx:
        tc = ctx.enter_context(TileContext(nc))
        const = ctx.enter_context(tc.tile_pool(name="const", bufs=1))

        aug_sb = const.tile([KAUG, AUG_W], fp16, name="aug_sb")
        nc.sync.dma_start(out=aug_sb, in_=aug[:, :])
        LC = aug_sb[:, 0:NUM_IN]
        LX = aug_sb[:, NUM_IN : 2 * NUM_IN]
        RC = aug_sb[:, 2 * NUM_IN : 2 * NUM_IN + O_SHARD]
        RX = aug_sb[:, 2 * NUM_IN + O_SHARD : AUG_W]

        bias_sb = const.tile([128, OT], fp32, name="bias_sb")
        nc.sync.dma_start(out=bias_sb, in_=bias[:, :])

        eps_sb = const.tile([128, 1], fp32, name="eps_sb")
        nc.vector.memset(eps_sb, 1e-6)

        # load x in bh-half order so the first batch-half's chunks land first
        x_sb = const.tile([128, K_TILES, BATCH], fp16, name="x_sb")
        for bh in range(BH):
            bsl = slice(bh * BG * 512, (bh + 1) * BG * 512)
            for kt in range(K_TILES):
                nc.sync.dma_start(out=x_sb[:, kt, bsl], in_=xT[kt][:, bsl])

        w_sb = const.tile([128, K_TILES, O_SHARD], fp16, name="w_sb")
        if "fakew" in variant:
            # timing-only: skip weight gen, fill w with a constant
            nc.vector.memset(w_sb, 0.01)

        for _rep in range(n_rep):
            _build_body(nc, tc, variant, LC, LX, RC, RX, bias_sb, eps_sb,
                        x_sb, w_sb, out, fp32, fp16)

    nc.finalize()
    if "stripldw" in variant:
        _strip_redundant_ldw(nc)
    return nc


def _strip_redundant_ldw(nc):
    """Remove InstLdweights that reload the exact weights already resident
    (same access pattern, no sync waits/updates, only matmuls/semaphores in
    between). Matmults are non-self-loading (ldweights=False) and keep using
    the loaded array contents."""
    n_del = 0
    for blk in nc.m.functions[0].blocks:
        keep = []
        last_sig = None
        for ins in blk.instructions:
            t = type(ins).__name__
            if t == "InstLdweights":
                sig = (str(ins.ins[0]), str(ins.perf_mode),
                       str(ins.is_transpose))
                si = ins.sync_info
                clean = si is None or (len(si.on_wait) == 0
                                       and len(si.on_update) == 0)
                if sig == last_sig and clean:
                    n_del += 1
                    continue
                last_sig = sig
            elif t in ("InstMatmult", "InstEventSemaphore"):
                pass
            else:
                last_sig = None
            keep.append(ins)
        if len(keep) != len(blk.instructions):
            blk.set_instructions(keep) if hasattr(blk, "set_instructions") \
                else _replace_block_instructions(blk, keep)
    return n_del


def _replace_block_instructions(blk, keep):
    il = blk.instructions
    for ins in [i for i in il if i not in keep]:
        il.remove(ins)


def _build_body(nc, tc, variant, LC, LX, RC, RX, bias_sb, eps_sb, x_sb,
                w_sb, out, fp32, fp16):
    import concourse.mybir as mybir

    LAG = 2  # wgen tiles of headroom before the interleaved group consumes

    def main_mms(grp, bh, ot, kt):
        lhsT = w_sb[:, kt, ot * 128 : (ot + 1) * 128]
        for g in range(BG):
            bsl = slice(bh * BG * 512 + g * 512, bh * BG * 512 + (g + 1) * 512)
            nc.tensor.matmul(grp[g], lhsT, x_sb[:, kt, bsl],
                             start=(kt == 0), stop=(kt == K_TILES - 1))

    def drain_group(opool, grp, bh, ot):
        if "nodrain" in variant:
            return
        osb = opool.tile([128, BG * 512], fp16, tag="osb", bufs=3, name="osb")
        for g in range(BG):
            # drain+store per bank so the last bank's DMA isn't gated on all 4
            nc.vector.tensor_scalar_add(
                osb[:, g * 512 : (g + 1) * 512], grp[g],
                bias_sb[:, ot : ot + 1])
            nc.sync.dma_start(
                out=out[ot * 128 : (ot + 1) * 128,
                        bh * BG * 512 + g * 512 : bh * BG * 512 + (g + 1) * 512],
                in_=osb[:, g * 512 : (g + 1) * 512])

    with tc.tile_pool(name="op", bufs=3) as opool:
        # --- weight gen, with main groups ot0/ot1 interleaved in its shadow ---
        # per (oh, kt): two K=21 fp16 matmuls land dC^2 | dX^2 in one 2-bank
        # PSUM tile; one ScalarE sqrt(x + 1e-6) (the bias replaces the >=0
        # clamp: true min dist^2 is 3e-5, hi/lo error ~1e-7); DVE does only
        # the final w = dC - dX subtract (fp16 into w_sb).
        if "nowgen" not in variant and "fakew" not in variant:
            with tc.tile_pool(name="wps", bufs=2, space="PSUM") as wps, \
                 tc.tile_pool(name="mps0", bufs=1, space="PSUM") as mps0, \
                 tc.tile_pool(name="wtmp", bufs=3) as wtmp:
                for oh in range(2):
                    osl = slice(oh * 512, (oh + 1) * 512)
                    ot = oh  # group ot0 rides oh0's shadow, ot1 rides oh1's
                    if "nomm" not in variant:
                        grp = [mps0.tile([128, 512], fp32, tag=f"g{g}",
                                         bufs=1, name=f"g{g}")
                               for g in range(BG)]
                    for kt in range(K_TILES):
                        ksl = slice(kt * 128, (kt + 1) * 128)
                        psCX = wps.tile([128, 1024], fp32, tag="psCX", bufs=2,
                                        name="psCX")
                        nc.tensor.matmul(psCX[:, 0:512], LC[:, ksl],
                                         RC[:, osl], start=True, stop=True)
                        nc.tensor.matmul(psCX[:, 512:1024], LX[:, ksl],
                                         RX[:, osl], start=True, stop=True)
                        t = wtmp.tile([128, 1024], fp32, tag="t", bufs=3,
                                      name="t")
                        nc.scalar.activation(
                            t, psCX, mybir.ActivationFunctionType.Sqrt,
                            bias=eps_sb[:, 0:1])
                        nc.vector.tensor_sub(w_sb[:, kt, osl], t[:, 0:512],
                                             t[:, 512:1024])
                        if "nomm" not in variant and kt >= LAG:
                            main_mms(grp, 0, ot, kt - LAG)
                    if "nomm" not in variant:
                        for kt in range(K_TILES - LAG, K_TILES):
                            main_mms(grp, 0, ot, kt)
                        drain_group(opool, grp, 0, ot)

        # --- remaining main groups: out[o,b] = sum_k w[k,o] x[k,b] (+bias) ---
        if "nomm" in variant:
            return
        first = (2 if ("nowgen" not in variant and "fakew" not in variant)
                 else 0)
        with tc.tile_pool(name="mps", bufs=2, space="PSUM") as mps:
            for bh in range(BH):
                for ot in range(first if bh == 0 else 0, OT):
                    grp = [mps.tile([128, 512], fp32, tag=f"ps{g}", bufs=2,
                                    name=f"ps{g}") for g in range(BG)]
                    for kt in range(K_TILES):
                        main_mms(grp, bh, ot, kt)
                    drain_group(opool, grp, bh, ot)


def _hilo(m):
    hi = m.astype(np.float16)
    lo = (m - hi.astype(np.float32)).astype(np.float16)
    return hi, lo


def _aug_pair(p, q):
    """dist^2(p_i, q_o) = sum_j L[j,i] * R[j,o], K=7 fp32 aug."""
    f32 = np.float32
    L = np.concatenate(
        [p.T, (p * p).sum(1)[None], np.ones((1, p.shape[0]), f32)], 0
    ).astype(f32)
    R = np.concatenate(
        [-2.0 * q.T, np.ones((1, q.shape[0]), f32), (q * q).sum(1)[None]], 0
    ).astype(f32)
    return L, R


def _hilo_big(L, R):
    """K=21 fp16 operands whose exact-product matmul reproduces fp32 L.T@R."""
    Lh, Ll = _hilo(L)
    Rh, Rl = _hilo(R)
    Lbig = np.concatenate([Lh, Lh, Ll], 0)
    Rbig = np.concatenate([Rh, Rl, Rh], 0)
    return Lbig, Rbig


def _prep_inputs(inputs, init_in_pos, init_out_pos, in_pos, out_pos, biases):
    f32 = np.float32
    x = np.asarray(inputs, dtype=f32)
    a = np.asarray(in_pos, dtype=f32).reshape(NUM_IN, SD)
    a0 = np.asarray(init_in_pos, dtype=f32).reshape(NUM_IN, SD)
    b = np.asarray(out_pos, dtype=f32).reshape(NUM_OUT, SD)
    b0 = np.asarray(init_out_pos, dtype=f32).reshape(NUM_OUT, SD)
    bias_full = np.asarray(biases, dtype=f32).reshape(NUM_OUT)

    # xT[kt, p, b] = x[b, kt*128+p]
    xT = np.ascontiguousarray(
        x.reshape(BATCH, K_TILES, 128).transpose(1, 2, 0).astype(np.float16)
    )

    LCf, RCf = _aug_pair(a, b)
    LXf, RXf = _aug_pair(a0, b0)
    LC, RC_full = _hilo_big(LCf, RCf)
    LX, RX_full = _hilo_big(LXf, RXf)

    in_maps = []
    for c in range(N_CORES):
        sl = slice(c * O_SHARD, (c + 1) * O_SHARD)
        aug = np.ascontiguousarray(
            np.concatenate([LC, LX, RC_full[:, sl], RX_full[:, sl]], axis=1)
        )
        bias_c = np.ascontiguousarray(
            bias_full[sl].reshape(OT, 128).T.astype(f32)
        )
        in_maps.append({"xT": xT, "aug": aug, "bias": bias_c})
    return in_maps


def _run(in_maps, trace=False, variant=""):
    from concourse.bass_utils import run_bass_kernel_spmd

    key = "nc" + variant
    if key not in _CACHE:
        _CACHE[key] = _build_bass(variant)
    nc = _CACHE[key]
    res = run_bass_kernel_spmd(
        nc, in_maps, core_ids=list(range(N_CORES)), trace=trace
    )
    full = np.empty((BATCH, NUM_OUT), dtype=np.float32)
    for c, r in enumerate(res.results):
        full[:, c * O_SHARD : (c + 1) * O_SHARD] = r["out"].T
    return full, res


DEFAULT_VARIANT = os.environ.get("MESHFC_VARIANT", "stripldw")


def kernel(**inputs) -> np.ndarray:
    in_maps = _prep_inputs(**inputs)
    out, _ = _run(in_maps, trace=bool(os.environ.get("MESHFC_TRACE")),
                  variant=DEFAULT_VARIANT)
    return out
